# revision 19
# baseline (speedup 1.0000x reference)
"""Multi-head attention (B=4, S=2048, D=1024, H=16) on 8 TRN2 NeuronCores. v4.

Sharding: core c = (batch c//2, seq-half c%2); K/V computed for the full
sequence on every core, Q/attention/outproj only for the core's half; host
rotates the sequence so the core's own half sits at columns [0, SH).
Everything on-chip is transposed (features on partitions): qT/kT [D,s],
scoresT [sk,sq], outT [d,sq].

v4 redesign vs v3 (562us):
- CP-everywhere masking: every sk chunk is patched to 1.0 at masked slots by
  one copy_predicated per 4 chunks (mask broadcast over the 2-head dup via a
  stride-0 AP). No mul-path, no mask-correction matmuls, no vsum/cnt.
- V bias folded into the output projection bias on host (bo' = bo + Wo@bv,
  uses sum(attn_weights)=1), so V psum->SBUF is a pure copy on DVE.
- Softmax denominator via the ones column in packed v (positive now).
- One persistent 4-bank PSUM ring [128,2048] shared by scores/proj/outproj/
  bcast; exps run at N=2048 (two chunks per ACTIVATE).
- Projections are emitted as PE filler INSIDE the attention loop (pair pr
  emits Q/K of pair pr+2; V is emitted during pair 0) so the PE never idles
  while ScalarE runs exp, and ScalarE starts exp'ing ~10us into the kernel.
- Per-(pr,sq) normalization: reciprocal of the Z row straight out of PSUM
  (bf16), broadcast matmul, and the final scale runs on GpSimd.
"""

import os
import sys
from collections import deque

import numpy as np

for _p in ("/opt/trn_rl_repo",):
    if _p not in sys.path and os.path.isdir(_p):
        sys.path.insert(0, _p)

import ml_dtypes

import concourse.bass as bass
from concourse import bacc
import concourse.mybir as mybir
import concourse.tile as tile
from concourse.bass_utils import run_bass_kernel_spmd

BF16 = mybir.dt.bfloat16
F32 = mybir.dt.float32
U8 = mybir.dt.uint8
AF = mybir.ActivationFunctionType
MULT = mybir.AluOpType.mult

B, S, D, H, DH = 4, 2048, 1024, 16, 64
NCORES = 8
SH = S // 2
P = 128
NB = 512
KC = D // P          # 8 feature chunks
SKC = S // P         # 16 sk chunks
NPAIR = H // 2       # 8 head pairs == feature chunks
SQB = SH // NB       # 2 q blocks per core
VW = DH + 1          # v columns + ones column
VROW = H * VW

_bf16 = ml_dtypes.bfloat16


def _build_bass():
    nc = bacc.Bacc(num_devices=NCORES)

    qT_d = nc.declare_dram_parameter("qT", [D, S], BF16, isOutput=False)
    mu_d = nc.declare_dram_parameter("mu", [S, SH], U8, isOutput=False)
    wqT_d = nc.declare_dram_parameter("wqT", [D, D], BF16, isOutput=False)
    wkT_d = nc.declare_dram_parameter("wkT", [D, D], BF16, isOutput=False)
    wvT_d = nc.declare_dram_parameter("wvT", [D, D], BF16, isOutput=False)
    woT_d = nc.declare_dram_parameter("woT", [D, D], BF16, isOutput=False)
    bq_d = nc.declare_dram_parameter("bq", [D, 1], F32, isOutput=False)
    bk_d = nc.declare_dram_parameter("bk", [D, 1], F32, isOutput=False)
    bo_d = nc.declare_dram_parameter("bo", [D, 1], F32, isOutput=False)
    sel_d = nc.declare_dram_parameter("sel", [2, P], BF16, isOutput=False)
    out_d = nc.declare_dram_parameter("out", [D, SH], F32, isOutput=True)

    with tile.TileContext(nc) as tc:
        with (
            tc.tile_pool(name="persist", bufs=1) as persist,
            tc.tile_pool(name="qk", bufs=3) as qkpool,
            tc.tile_pool(name="apool", bufs=2) as apool,
            tc.tile_pool(name="work", bufs=2) as work,
            tc.tile_pool(name="psP", bufs=1, space="PSUM") as psP,
            tc.tile_pool(name="psV", bufs=2, space="PSUM") as psV,
        ):
            # ---------------- persistent SBUF ----------------
            qTb = persist.tile([P, KC * S], BF16)
            for kc in range(KC):
                nc.sync.dma_start(qTb[:, kc * S:(kc + 1) * S], qT_d[kc * P:(kc + 1) * P, :])
            wq = persist.tile([P, KC * D], BF16)
            wk = persist.tile([P, KC * D], BF16)
            wv = persist.tile([P, KC * D], BF16)
            wo = persist.tile([P, KC * D], BF16)
            for kc in range(KC):
                nc.sync.dma_start(wq[:, kc * D:(kc + 1) * D], wqT_d[kc * P:(kc + 1) * P, :])
                nc.sync.dma_start(wk[:, kc * D:(kc + 1) * D], wkT_d[kc * P:(kc + 1) * P, :])
                nc.sync.dma_start(wv[:, kc * D:(kc + 1) * D], wvT_d[kc * P:(kc + 1) * P, :])
                nc.sync.dma_start(wo[:, kc * D:(kc + 1) * D], woT_d[kc * P:(kc + 1) * P, :])
            bq_sb = persist.tile([P, KC], F32)
            bk_sb = persist.tile([P, KC], F32)
            bo_sb = persist.tile([P, KC], F32)
            for mc in range(KC):
                nc.sync.dma_start(bq_sb[:, mc:mc + 1], bq_d[mc * P:(mc + 1) * P, :])
                nc.sync.dma_start(bk_sb[:, mc:mc + 1], bk_d[mc * P:(mc + 1) * P, :])
                nc.sync.dma_start(bo_sb[:, mc:mc + 1], bo_d[mc * P:(mc + 1) * P, :])
            sel_sb = persist.tile([2, P], BF16)
            nc.sync.dma_start(sel_sb[:], sel_d[:])
            recf = persist.tile([2, NB], BF16)
            nc.any.memset(recf[:], 1.0)
            sums = persist.tile([2, NB], BF16)
            nc.any.memset(sums[:], 1.0)
            # inverted mask, [sk-part within chunk, (sq, chunk, q)]
            mTu = persist.tile([P, SQB * SKC * NB], U8)
            for sq in range(SQB):
                for c in range(SKC):
                    nc.sync.dma_start(
                        mTu[:, (sq * SKC + c) * NB:(sq * SKC + c + 1) * NB],
                        mu_d[c * P:(c + 1) * P, sq * NB:(sq + 1) * NB],
                    )
            ones1 = persist.tile([P, 4], BF16)
            nc.any.memset(ones1[:], 1.0)
            vpk = persist.tile([P, SKC * VROW], BF16)
            nc.any.memset(vpk[:], 1.0)   # ones columns; v parts overwritten
            outMT = persist.tile([P, KC * SH], BF16)

            # one 4-bank psum ring; slot == [128, 1024] half, strict alternation
            psBig = psP.tile([P, 4 * NB], F32)
            _ps = {"i": 0}

            def ps_half():
                h = _ps["i"] % 2
                _ps["i"] += 1
                return psBig[:, h * 2 * NB:(h + 1) * 2 * NB], h

            def ps_align():
                if _ps["i"] % 2 == 1:
                    _ps["i"] += 1

            # ---------------- projection emitters ----------------
            def emit_q(pr):
                t = qkpool.tile([P, SH], BF16, tag="qT", name="qTp")
                ps, _h = ps_half()
                for kc in range(KC):
                    w_sl = wq[:, kc * D + pr * P: kc * D + (pr + 1) * P]
                    nc.tensor.matmul(ps[:, 0:NB], w_sl, qTb[:, kc * S: kc * S + NB],
                                     start=(kc == 0), stop=(kc == KC - 1))
                    nc.tensor.matmul(ps[:, NB:2 * NB], w_sl, qTb[:, kc * S + NB: kc * S + 2 * NB],
                                     start=(kc == 0), stop=(kc == KC - 1))
                nc.scalar.activation(t[:], ps[:], AF.Identity, bias=bq_sb[:, pr:pr + 1])
                return t

            def emit_k_half(pr, t, nbp):
                ps, _h = ps_half()
                for kc in range(KC):
                    w_sl = wk[:, kc * D + pr * P: kc * D + (pr + 1) * P]
                    base = kc * S + nbp * 2 * NB
                    nc.tensor.matmul(ps[:, 0:NB], w_sl, qTb[:, base: base + NB],
                                     start=(kc == 0), stop=(kc == KC - 1))
                    nc.tensor.matmul(ps[:, NB:2 * NB], w_sl, qTb[:, base + NB: base + 2 * NB],
                                     start=(kc == 0), stop=(kc == KC - 1))
                nc.scalar.activation(t[:, nbp * 2 * NB:(nbp + 1) * 2 * NB], ps[:],
                                     AF.Identity, bias=bk_sb[:, pr:pr + 1])

            def emit_v(sc):
                ps, _h = ps_half()
                for kc in range(KC):
                    x_sl = qTb[:, kc * S + sc * P: kc * S + (sc + 1) * P]
                    nc.tensor.matmul(ps[:, 0:NB], x_sl, wv[:, kc * D: kc * D + NB],
                                     start=(kc == 0), stop=(kc == KC - 1))
                    nc.tensor.matmul(ps[:, NB:2 * NB], x_sl, wv[:, kc * D + NB: kc * D + 2 * NB],
                                     start=(kc == 0), stop=(kc == KC - 1))
                vdst3 = vpk[:, sc * VROW:(sc + 1) * VROW].rearrange("p (h w) -> p h w", h=H)
                nc.vector.tensor_copy(vdst3[:, :, 0:DH], ps.rearrange("p (h w) -> p h w", h=H))

            # filler queue: (min_pair, closure).  Entries for pair pr may only
            # be emitted from pair pr-2 on (qk ring bufs=3 -> the ACT eviction
            # wait must target an already-finished pair).
            qts, kts = {}, {}
            filler = deque()
            for sc in range(SKC):
                filler.append((0, lambda sc=sc: emit_v(sc)))

            def queue_qk(pr):
                def do_q(pr=pr):
                    qts[pr] = emit_q(pr)
                def do_k0(pr=pr):
                    kts[pr] = qkpool.tile([P, S], BF16, tag="kT", name="kTp")
                    emit_k_half(pr, kts[pr], 0)
                def do_k1(pr=pr):
                    emit_k_half(pr, kts[pr], 1)
                filler.append((max(0, pr - 2), do_q))
                filler.append((max(0, pr - 2), do_k0))
                filler.append((max(0, pr - 2), do_k1))

            for pr in range(1, NPAIR):
                queue_qk(pr)

            def pop_filler(cur_pair, n):
                done = 0
                while done < n and filler and filler[0][0] <= cur_pair:
                    filler.popleft()[1]()
                    done += 1
                if done % 2 == 1:
                    ps_half()   # keep psum half parity for paired exps

            # prologue: pair 0 projections
            qts[0] = emit_q(0)
            kts[0] = qkpool.tile([P, S], BF16, tag="kT", name="kTp")
            emit_k_half(0, kts[0], 0)
            emit_k_half(0, kts[0], 1)

            # ---------------- attention ----------------
            for pr in range(NPAIR):
                qt, kt = qts[pr], kts[pr]
                for sq in range(SQB):
                    pv0 = psV.tile([P, NB], F32, tag="pv0")
                    pv1 = psV.tile([P, NB], F32, tag="pv1")

                    def emit_pv(g, a01g):
                        for c4 in range(4):
                            c = 4 * g + c4
                            for h01, pv in ((0, pv0), (1, pv1)):
                                hloc = 2 * pr + h01
                                nc.tensor.matmul(
                                    pv[0:VW, :],
                                    vpk[:, c * VROW + hloc * VW: c * VROW + (hloc + 1) * VW],
                                    a01g[:, c4 * 2 * NB + h01 * NB: c4 * 2 * NB + (h01 + 1) * NB],
                                    start=(c == 0), stop=(c == SKC - 1),
                                )

                    prev = None
                    for g in range(4):
                        a01g = apool.tile([P, 4 * 2 * NB], BF16, tag="a01")
                        for cp2 in range(2):
                            ps_align()
                            for cc in range(2):
                                c = 4 * g + 2 * cp2 + cc
                                ps, h = ps_half()
                                nc.tensor.matmul(
                                    ps[:, 0:NB], kt[0:DH, c * P:(c + 1) * P],
                                    qt[0:DH, sq * NB:(sq + 1) * NB],
                                    start=True, stop=True, tile_position=(0, 0),
                                )
                                nc.tensor.matmul(
                                    ps[:, NB:2 * NB], kt[DH:P, c * P:(c + 1) * P],
                                    qt[DH:P, sq * NB:(sq + 1) * NB],
                                    start=True, stop=True, tile_position=(64, 0),
                                )
                            # exp over both chunks in one ACTIVATE (N=2048)
                            nc.scalar.activation(
                                a01g[:, cp2 * 2 * 2 * NB:(cp2 + 1) * 2 * 2 * NB],
                                psBig[:], AF.Exp, scale=0.125,
                            )
                        # patch masked slots to 1.0 (mask broadcast over head dup)
                        msl = mTu[:, (sq * SKC + 4 * g) * NB:(sq * SKC + 4 * g + 4) * NB]
                        m4 = msl.rearrange("p (c q) -> p c q", c=4).unsqueeze(2) \
                            .broadcast_to([P, 4, 2, NB])
                        o4 = ones1[:, 0:1].unsqueeze(2).unsqueeze(3) \
                            .broadcast_to([P, 4, 2, NB])
                        nc.vector.copy_predicated(
                            a01g[:].rearrange("p (c d q) -> p c d q", c=4, d=2),
                            m4, o4,
                        )
                        if prev is not None:
                            emit_pv(*prev)
                        pop_filler(pr, 4 if (pr == 0 and sq == 0) else 2)
                        prev = (g, a01g)
                    emit_pv(*prev)

                    # -------- normalization of this (pr, sq) --------
                    za = work.tile([P, NB], BF16, tag="za")
                    zb = work.tile([P, NB], BF16, tag="zb")
                    nc.scalar.copy(za[0:VW, :], pv0[0:VW, :])
                    nc.scalar.copy(zb[0:VW, :], pv1[0:VW, :])
                    nc.sync.dma_start(sums[0:1, :], za[DH:VW, :])
                    nc.sync.dma_start(sums[1:2, :], zb[DH:VW, :])
                    with nc.allow_low_precision("bf16 softmax denominators"):
                        nc.vector.reciprocal(recf[:], sums[:])
                    bcp, _h = ps_half()
                    ps_half()  # bcast uses one half; burn the partner for parity
                    nc.tensor.matmul(bcp[0:DH, 0:NB], sel_sb[:, 0:DH], recf[:],
                                     start=True, stop=True)
                    nc.tensor.matmul(bcp[0:DH, NB:2 * NB], sel_sb[:, DH:P], recf[:],
                                     start=True, stop=True)
                    bc_sb = work.tile([P, 2 * NB], BF16, tag="bc", bufs=1)
                    nc.scalar.copy(bc_sb[0:DH, :], bcp[0:DH, :])
                    od = pr * SH + sq * NB
                    nc.gpsimd.tensor_mul(
                        outMT[0:DH, od:od + NB], za[0:DH, :], bc_sb[0:DH, 0:NB])
                    nc.gpsimd.tensor_mul(
                        outMT[DH:P, od:od + NB], zb[0:DH, :], bc_sb[0:DH, NB:2 * NB])

            # ---------------- output projection ----------------
            for mc in range(KC):
                ps, _h = ps_half()
                for kc in range(KC):
                    w_sl = wo[:, kc * D + mc * P: kc * D + (mc + 1) * P]
                    nc.tensor.matmul(ps[:, 0:NB], w_sl, outMT[:, kc * SH: kc * SH + NB],
                                     start=(kc == 0), stop=(kc == KC - 1))
                    nc.tensor.matmul(ps[:, NB:2 * NB], w_sl,
                                     outMT[:, kc * SH + NB: kc * SH + 2 * NB],
                                     start=(kc == 0), stop=(kc == KC - 1))
                for nb in range(2):
                    fin = work.tile([P, NB], F32, tag="fin", bufs=2)
                    nc.scalar.activation(fin[:], ps[:, nb * NB:(nb + 1) * NB],
                                         AF.Identity, bias=bo_sb[:, mc:mc + 1])
                    nc.sync.dma_start(out_d[mc * P:(mc + 1) * P, nb * NB:(nb + 1) * NB],
                                      fin[:])

    nc.finalize()
    return nc


_NC_CACHE = None
LAST_RESULTS = None


def _get_nc():
    global _NC_CACHE
    if _NC_CACHE is None:
        _NC_CACHE = _build_bass()
    return _NC_CACHE


def kernel(query, mask, Wq, bq, Wk, bk, Wv, bv, Wo, bo, **_unused):
    query = np.asarray(query, dtype=np.float32)
    mask = np.asarray(mask).astype(bool)
    Wq = np.asarray(Wq, dtype=np.float32)
    Wk = np.asarray(Wk, dtype=np.float32)
    Wv = np.asarray(Wv, dtype=np.float32)
    Wo = np.asarray(Wo, dtype=np.float32)
    bq = np.asarray(bq, dtype=np.float32)
    bk = np.asarray(bk, dtype=np.float32)
    bv = np.asarray(bv, dtype=np.float32)
    bo = np.asarray(bo, dtype=np.float32)

    wqT = np.ascontiguousarray(Wq.T).astype(_bf16)
    wkT = np.ascontiguousarray(Wk.T).astype(_bf16)
    wvT = np.ascontiguousarray(Wv.T).astype(_bf16)
    woT = np.ascontiguousarray(Wo.T).astype(_bf16)
    bq_c = np.ascontiguousarray(bq.reshape(D, 1))
    bk_c = np.ascontiguousarray(bk.reshape(D, 1))
    # V bias folded through the output projection (sum of attn weights == 1)
    bo_c = np.ascontiguousarray((bo + Wo @ bv).reshape(D, 1))
    sel_np = np.zeros((2, P), dtype=np.float32)
    sel_np[0, 0:DH] = 1.0      # head-0 selector: stationary cols 0:64
    sel_np[1, DH:P] = 1.0      # head-1 selector: stationary cols 64:128
    sel_bf = sel_np.astype(_bf16)

    in_maps = []
    for c in range(NCORES):
        b, half = c // 2, c % 2
        off = half * SH
        qT_rot = np.ascontiguousarray(np.roll(query[b].T, -off, axis=1)).astype(_bf16)
        minv = np.roll((~mask[b]).T, -off, axis=0)      # [sk, q], True where masked
        mu8 = np.ascontiguousarray(minv[:, off:off + SH]).astype(np.uint8)
        in_maps.append({
            "qT": qT_rot, "mu": mu8,
            "wqT": wqT, "wkT": wkT, "wvT": wvT, "woT": woT,
            "bq": bq_c, "bk": bk_c, "bo": bo_c, "sel": sel_bf,
            "out": np.zeros((D, SH), dtype=np.float32),
        })

    nc = _get_nc()
    res = run_bass_kernel_spmd(nc, in_maps, core_ids=list(range(NCORES)))
    global LAST_RESULTS
    LAST_RESULTS = res

    out = np.empty((B, S, D), dtype=np.float32)
    for c in range(NCORES):
        b, half = c // 2, c % 2
        out[b, half * SH:(half + 1) * SH, :] = res.results[c]["out"].T
    return out


# revision 23
# speedup vs baseline: 1.2820x; 1.2820x over previous
"""Multi-head attention (B=4, S=2048, D=1024, H=16) on 8 TRN2 NeuronCores. v4.

Sharding: core c = (batch c//2, seq-half c%2); K/V computed for the full
sequence on every core, Q/attention/outproj only for the core's half; host
rotates the sequence so the core's own half sits at columns [0, SH).
Everything on-chip is transposed (features on partitions): qT/kT [D,s],
scoresT [sk,sq], outT [d,sq].

v4 redesign vs v3 (562us):
- CP-everywhere masking: every sk chunk is patched to 1.0 at masked slots by
  one copy_predicated per 4 chunks (mask broadcast over the 2-head dup via a
  stride-0 AP). No mul-path, no mask-correction matmuls, no vsum/cnt.
- V bias folded into the output projection bias on host (bo' = bo + Wo@bv,
  uses sum(attn_weights)=1), so V psum->SBUF is a pure copy on DVE.
- Softmax denominator via the ones column in packed v (positive now).
- One persistent 4-bank PSUM ring [128,2048] shared by scores/proj/outproj/
  bcast; exps run at N=2048 (two chunks per ACTIVATE).
- Projections are emitted as PE filler INSIDE the attention loop (pair pr
  emits Q/K of pair pr+2; V is emitted during pair 0) so the PE never idles
  while ScalarE runs exp, and ScalarE starts exp'ing ~10us into the kernel.
- Per-(pr,sq) normalization: reciprocal of the Z row straight out of PSUM
  (bf16), broadcast matmul, and the final scale runs on GpSimd.
"""

import os
import sys
from collections import deque

import numpy as np

for _p in ("/opt/trn_rl_repo",):
    if _p not in sys.path and os.path.isdir(_p):
        sys.path.insert(0, _p)

import ml_dtypes

import concourse.bass as bass
from concourse import bacc
import concourse.mybir as mybir
import concourse.tile as tile
from concourse.bass_utils import run_bass_kernel_spmd

BF16 = mybir.dt.bfloat16
F32 = mybir.dt.float32
U8 = mybir.dt.uint8
AF = mybir.ActivationFunctionType
MULT = mybir.AluOpType.mult

B, S, D, H, DH = 4, 2048, 1024, 16, 64
NCORES = 8
SH = S // 2
P = 128
NB = 512
KC = D // P          # 8 feature chunks
SKC = S // P         # 16 sk chunks
NPAIR = H // 2       # 8 head pairs == feature chunks
SQB = SH // NB       # 2 q blocks per core
VW = DH + 1          # v columns + ones column
VROW = H * VW

_bf16 = ml_dtypes.bfloat16


def _build_bass():
    nc = bacc.Bacc(num_devices=NCORES)

    qT_d = nc.declare_dram_parameter("qT", [D, S], BF16, isOutput=False)
    mu_d = nc.declare_dram_parameter("mu", [S, SH], U8, isOutput=False)
    wqT_d = nc.declare_dram_parameter("wqT", [D, D], BF16, isOutput=False)
    wkT_d = nc.declare_dram_parameter("wkT", [D, D], BF16, isOutput=False)
    wvT_d = nc.declare_dram_parameter("wvT", [D, D], BF16, isOutput=False)
    woT_d = nc.declare_dram_parameter("woT", [D, D], BF16, isOutput=False)
    bq_d = nc.declare_dram_parameter("bq", [D, 1], F32, isOutput=False)
    bk_d = nc.declare_dram_parameter("bk", [D, 1], F32, isOutput=False)
    bo_d = nc.declare_dram_parameter("bo", [D, 1], F32, isOutput=False)
    sel_d = nc.declare_dram_parameter("sel", [2, P], BF16, isOutput=False)
    out_d = nc.declare_dram_parameter("out", [D, SH], F32, isOutput=True)

    with tile.TileContext(nc) as tc:
        with (
            tc.tile_pool(name="persist", bufs=1) as persist,
            tc.tile_pool(name="qk", bufs=3) as qkpool,
            tc.tile_pool(name="apool", bufs=2) as apool,
            tc.tile_pool(name="work", bufs=2) as work,
            tc.tile_pool(name="psS", bufs=2, space="PSUM") as psS,
            tc.tile_pool(name="psV", bufs=2, space="PSUM") as psV,
        ):
            # ---------------- persistent SBUF ----------------
            qTb = persist.tile([P, KC * S], BF16)
            for kc in range(KC):
                nc.sync.dma_start(qTb[:, kc * S:(kc + 1) * S], qT_d[kc * P:(kc + 1) * P, :])
            wq = persist.tile([P, KC * D], BF16)
            wk = persist.tile([P, KC * D], BF16)
            wv = persist.tile([P, KC * D], BF16)
            for kc in range(KC):
                nc.sync.dma_start(wq[:, kc * D:(kc + 1) * D], wqT_d[kc * P:(kc + 1) * P, :])
                nc.sync.dma_start(wk[:, kc * D:(kc + 1) * D], wkT_d[kc * P:(kc + 1) * P, :])
                nc.sync.dma_start(wv[:, kc * D:(kc + 1) * D], wvT_d[kc * P:(kc + 1) * P, :])
            bq_sb = persist.tile([P, KC], F32)
            bk_sb = persist.tile([P, KC], F32)
            bo_sb = persist.tile([P, KC], F32)
            for mc in range(KC):
                nc.sync.dma_start(bq_sb[:, mc:mc + 1], bq_d[mc * P:(mc + 1) * P, :])
                nc.sync.dma_start(bk_sb[:, mc:mc + 1], bk_d[mc * P:(mc + 1) * P, :])
                nc.sync.dma_start(bo_sb[:, mc:mc + 1], bo_d[mc * P:(mc + 1) * P, :])
            sel_sb = persist.tile([2, P], BF16)
            nc.sync.dma_start(sel_sb[:], sel_d[:])
            recf = persist.tile([2, NB], BF16)
            nc.any.memset(recf[:], 1.0)
            sums = persist.tile([2, NB], F32)
            nc.any.memset(sums[:], 1.0)
            # inverted mask, [sk-part within chunk, (sq, chunk, q)]
            mTu = persist.tile([P, SQB * SKC * NB], U8)
            for sq in range(SQB):
                for c in range(SKC):
                    nc.sync.dma_start(
                        mTu[:, (sq * SKC + c) * NB:(sq * SKC + c + 1) * NB],
                        mu_d[c * P:(c + 1) * P, sq * NB:(sq + 1) * NB],
                    )
            ones1 = persist.tile([P, 4], BF16)
            nc.any.memset(ones1[:], 1.0)
            vpk = persist.tile([P, SKC * VROW], BF16)
            nc.any.memset(vpk[:], 1.0)   # ones columns; v parts overwritten
            outMT = persist.tile([P, KC * SH], BF16)

            def ps_tile():
                t = psS.tile([P, 2 * NB], F32, tag="ps", name="ps")
                return t

            # ---------------- projection emitters ----------------
            def emit_q(pr):
                t = qkpool.tile([P, SH], BF16, tag="qT", name="qTp")
                ps = ps_tile()
                for kc in range(KC):
                    w_sl = wq[:, kc * D + pr * P: kc * D + (pr + 1) * P]
                    nc.tensor.matmul(ps[:, 0:NB], w_sl, qTb[:, kc * S: kc * S + NB],
                                     start=(kc == 0), stop=(kc == KC - 1))
                    nc.tensor.matmul(ps[:, NB:2 * NB], w_sl, qTb[:, kc * S + NB: kc * S + 2 * NB],
                                     start=(kc == 0), stop=(kc == KC - 1))
                nc.scalar.activation(t[:], ps[:], AF.Identity, bias=bq_sb[:, pr:pr + 1])
                return t

            def emit_k_half(pr, t, nbp):
                ps = ps_tile()
                for kc in range(KC):
                    w_sl = wk[:, kc * D + pr * P: kc * D + (pr + 1) * P]
                    base = kc * S + nbp * 2 * NB
                    nc.tensor.matmul(ps[:, 0:NB], w_sl, qTb[:, base: base + NB],
                                     start=(kc == 0), stop=(kc == KC - 1))
                    nc.tensor.matmul(ps[:, NB:2 * NB], w_sl, qTb[:, base + NB: base + 2 * NB],
                                     start=(kc == 0), stop=(kc == KC - 1))
                nc.scalar.activation(t[:, nbp * 2 * NB:(nbp + 1) * 2 * NB], ps[:],
                                     AF.Identity, bias=bk_sb[:, pr:pr + 1])

            def emit_v(sc):
                ps = ps_tile()
                for kc in range(KC):
                    x_sl = qTb[:, kc * S + sc * P: kc * S + (sc + 1) * P]
                    nc.tensor.matmul(ps[:, 0:NB], x_sl, wv[:, kc * D: kc * D + NB],
                                     start=(kc == 0), stop=(kc == KC - 1))
                    nc.tensor.matmul(ps[:, NB:2 * NB], x_sl, wv[:, kc * D + NB: kc * D + 2 * NB],
                                     start=(kc == 0), stop=(kc == KC - 1))
                vdst3 = vpk[:, sc * VROW:(sc + 1) * VROW].rearrange("p (h w) -> p h w", h=H)
                eng = nc.vector if sc % 2 == 0 else nc.scalar
                if sc % 2 == 0:
                    nc.vector.tensor_copy(vdst3[:, :, 0:DH], ps.rearrange("p (h w) -> p h w", h=H))
                else:
                    nc.scalar.copy(vdst3[:, :, 0:DH], ps.rearrange("p (h w) -> p h w", h=H))

            # filler queue: (min_pair, closure).  Entries for pair pr may only
            # be emitted from pair pr-2 on (qk ring bufs=3 -> the ACT eviction
            # wait must target an already-finished pair).
            qts, kts = {}, {}
            filler = deque()
            for sc in range(SKC):
                filler.append((0, lambda sc=sc: emit_v(sc)))

            def queue_qk(pr):
                def do_q(pr=pr):
                    qts[pr] = emit_q(pr)
                def do_k0(pr=pr):
                    kts[pr] = qkpool.tile([P, S], BF16, tag="kT", name="kTp")
                    emit_k_half(pr, kts[pr], 0)
                def do_k1(pr=pr):
                    emit_k_half(pr, kts[pr], 1)
                filler.append((max(0, pr - 2), do_q))
                filler.append((max(0, pr - 2), do_k0))
                filler.append((max(0, pr - 2), do_k1))

            for pr in range(1, NPAIR):
                queue_qk(pr)

            def pop_filler(cur_pair, n):
                done = 0
                while done < n and filler and filler[0][0] <= cur_pair:
                    filler.popleft()[1]()
                    done += 1

            # prologue: pair 0 projections
            qts[0] = emit_q(0)
            kts[0] = qkpool.tile([P, S], BF16, tag="kT", name="kTp")
            emit_k_half(0, kts[0], 0)
            emit_k_half(0, kts[0], 1)

            # ---------------- attention ----------------
            for pr in range(NPAIR):
                qt, kt = qts[pr], kts[pr]
                for sq in range(SQB):
                    pv0 = psV.tile([P, NB], F32, tag="pv0")
                    pv1 = psV.tile([P, NB], F32, tag="pv1")

                    def emit_pv(g, a01g):
                        for c4 in range(4):
                            c = 4 * g + c4
                            for h01, pv in ((0, pv0), (1, pv1)):
                                hloc = 2 * pr + h01
                                nc.tensor.matmul(
                                    pv[0:VW, :],
                                    vpk[:, c * VROW + hloc * VW: c * VROW + (hloc + 1) * VW],
                                    a01g[:, c4 * 2 * NB + h01 * NB: c4 * 2 * NB + (h01 + 1) * NB],
                                    start=(c == 0), stop=(c == SKC - 1),
                                )

                    prev = None
                    for g in range(4):
                        a01g = apool.tile([P, 4 * 2 * NB], BF16, tag="a01")
                        for c4 in range(4):
                            c = 4 * g + c4
                            ps = ps_tile()
                            nc.tensor.matmul(
                                ps[:, 0:NB], kt[0:DH, c * P:(c + 1) * P],
                                qt[0:DH, sq * NB:(sq + 1) * NB],
                                start=True, stop=True, tile_position=(0, 0),
                            )
                            nc.tensor.matmul(
                                ps[:, NB:2 * NB], kt[DH:P, c * P:(c + 1) * P],
                                qt[DH:P, sq * NB:(sq + 1) * NB],
                                start=True, stop=True, tile_position=(64, 0),
                            )
                            nc.scalar.activation(
                                a01g[:, c4 * 2 * NB:(c4 + 1) * 2 * NB],
                                ps[:], AF.Exp, scale=0.125,
                            )
                        # patch masked slots to 1.0 (mask broadcast over head dup)
                        msl = mTu[:, (sq * SKC + 4 * g) * NB:(sq * SKC + 4 * g + 4) * NB]
                        m4 = msl.rearrange("p (c q) -> p c q", c=4).unsqueeze(2) \
                            .broadcast_to([P, 4, 2, NB])
                        o4 = ones1[:, 0:1].unsqueeze(2).unsqueeze(3) \
                            .broadcast_to([P, 4, 2, NB])
                        nc.vector.copy_predicated(
                            a01g[:].rearrange("p (c d q) -> p c d q", c=4, d=2),
                            m4, o4,
                        )
                        if prev is not None:
                            emit_pv(*prev)
                        pop_filler(pr, 4 if (pr == 0 and sq == 0) else 2)
                        prev = (g, a01g)
                    emit_pv(*prev)

                    # -------- normalization of this (pr, sq) --------
                    za = work.tile([P, NB], BF16, tag="za")
                    zb = work.tile([P, NB], BF16, tag="zb")
                    zfa = work.tile([P, NB], F32, tag="zfa", bufs=1)
                    nc.scalar.copy(za[0:VW, :], pv0[0:VW, :])
                    nc.scalar.copy(zb[0:VW, :], pv1[0:VW, :])
                    nc.vector.tensor_copy(zfa[DH:VW, :], pv0[DH:VW, :])
                    nc.vector.tensor_copy(zfa[96:97, :], pv1[DH:VW, :])
                    nc.sync.dma_start(sums[0:1, :], zfa[DH:VW, :])
                    nc.sync.dma_start(sums[1:2, :], zfa[96:97, :])
                    nc.vector.reciprocal_approx_fast(zfa[0:2, :], sums[:])
                    with nc.allow_low_precision("bf16 softmax denominators"):
                        nc.vector.tensor_copy(recf[:], zfa[0:2, :])
                    bcp = ps_tile()
                    nc.tensor.matmul(bcp[0:DH, 0:NB], sel_sb[:, 0:DH], recf[:],
                                     start=True, stop=True)
                    nc.tensor.matmul(bcp[0:DH, NB:2 * NB], sel_sb[:, DH:P], recf[:],
                                     start=True, stop=True)
                    bc_sb = work.tile([P, 2 * NB], BF16, tag="bc", bufs=1)
                    nc.scalar.copy(bc_sb[0:DH, :], bcp[0:DH, :])
                    od = pr * SH + sq * NB
                    nc.gpsimd.tensor_mul(
                        outMT[0:DH, od:od + NB], za[0:DH, :], bc_sb[0:DH, 0:NB])
                    nc.gpsimd.tensor_mul(
                        outMT[DH:P, od:od + NB], zb[0:DH, :], bc_sb[0:DH, NB:2 * NB])

            # ---------------- output projection ----------------
            wo_rings = []
            for mc in range(KC):
                wo_mc = work.tile([P, KC * P], BF16, tag="womc", bufs=3, name="wo_mc")
                for kc in range(KC):
                    nc.sync.dma_start(wo_mc[:, kc * P:(kc + 1) * P],
                                      woT_d[kc * P:(kc + 1) * P, mc * P:(mc + 1) * P])
                wo_rings.append(wo_mc)
            for mc in range(KC):
                wo_mc = wo_rings[mc]
                ps = ps_tile()
                for kc in range(KC):
                    w_sl = wo_mc[:, kc * P:(kc + 1) * P]
                    nc.tensor.matmul(ps[:, 0:NB], w_sl, outMT[:, kc * SH: kc * SH + NB],
                                     start=(kc == 0), stop=(kc == KC - 1))
                    nc.tensor.matmul(ps[:, NB:2 * NB], w_sl,
                                     outMT[:, kc * SH + NB: kc * SH + 2 * NB],
                                     start=(kc == 0), stop=(kc == KC - 1))
                for nb in range(2):
                    fin = work.tile([P, NB], F32, tag="fin", bufs=1)
                    nc.scalar.activation(fin[:], ps[:, nb * NB:(nb + 1) * NB],
                                         AF.Identity, bias=bo_sb[:, mc:mc + 1])
                    nc.sync.dma_start(out_d[mc * P:(mc + 1) * P, nb * NB:(nb + 1) * NB],
                                      fin[:])

    nc.finalize()
    return nc


_NC_CACHE = None
LAST_RESULTS = None


def _get_nc():
    global _NC_CACHE
    if _NC_CACHE is None:
        _NC_CACHE = _build_bass()
    return _NC_CACHE


def kernel(query, mask, Wq, bq, Wk, bk, Wv, bv, Wo, bo, **_unused):
    query = np.asarray(query, dtype=np.float32)
    mask = np.asarray(mask).astype(bool)
    Wq = np.asarray(Wq, dtype=np.float32)
    Wk = np.asarray(Wk, dtype=np.float32)
    Wv = np.asarray(Wv, dtype=np.float32)
    Wo = np.asarray(Wo, dtype=np.float32)
    bq = np.asarray(bq, dtype=np.float32)
    bk = np.asarray(bk, dtype=np.float32)
    bv = np.asarray(bv, dtype=np.float32)
    bo = np.asarray(bo, dtype=np.float32)

    wqT = np.ascontiguousarray(Wq.T).astype(_bf16)
    wkT = np.ascontiguousarray(Wk.T).astype(_bf16)
    wvT = np.ascontiguousarray(Wv.T).astype(_bf16)
    woT = np.ascontiguousarray(Wo.T).astype(_bf16)
    bq_c = np.ascontiguousarray(bq.reshape(D, 1))
    bk_c = np.ascontiguousarray(bk.reshape(D, 1))
    # V bias folded through the output projection (sum of attn weights == 1)
    bo_c = np.ascontiguousarray((bo + Wo @ bv).reshape(D, 1))
    sel_np = np.zeros((2, P), dtype=np.float32)
    sel_np[0, 0:DH] = 1.0      # head-0 selector: stationary cols 0:64
    sel_np[1, DH:P] = 1.0      # head-1 selector: stationary cols 64:128
    sel_bf = sel_np.astype(_bf16)

    in_maps = []
    for c in range(NCORES):
        b, half = c // 2, c % 2
        off = half * SH
        qT_rot = np.ascontiguousarray(np.roll(query[b].T, -off, axis=1)).astype(_bf16)
        minv = np.roll((~mask[b]).T, -off, axis=0)      # [sk, q], True where masked
        mu8 = np.ascontiguousarray(minv[:, off:off + SH]).astype(np.uint8)
        in_maps.append({
            "qT": qT_rot, "mu": mu8,
            "wqT": wqT, "wkT": wkT, "wvT": wvT, "woT": woT,
            "bq": bq_c, "bk": bk_c, "bo": bo_c, "sel": sel_bf,
            "out": np.zeros((D, SH), dtype=np.float32),
        })

    nc = _get_nc()
    res = run_bass_kernel_spmd(nc, in_maps, core_ids=list(range(NCORES)))
    global LAST_RESULTS
    LAST_RESULTS = res

    out = np.empty((B, S, D), dtype=np.float32)
    for c in range(NCORES):
        b, half = c // 2, c % 2
        out[b, half * SH:(half + 1) * SH, :] = res.results[c]["out"].T
    return out


# revision 24
# speedup vs baseline: 1.5175x; 1.1837x over previous
"""Multi-head attention (B=4, S=2048, D=1024, H=16) on 8 TRN2 NeuronCores. v4.

Sharding: core c = (batch c//2, seq-half c%2); K/V computed for the full
sequence on every core, Q/attention/outproj only for the core's half; host
rotates the sequence so the core's own half sits at columns [0, SH).
Everything on-chip is transposed (features on partitions): qT/kT [D,s],
scoresT [sk,sq], outT [d,sq].

v4 redesign vs v3 (562us):
- CP-everywhere masking: every sk chunk is patched to 1.0 at masked slots by
  one copy_predicated per 4 chunks (mask broadcast over the 2-head dup via a
  stride-0 AP). No mul-path, no mask-correction matmuls, no vsum/cnt.
- V bias folded into the output projection bias on host (bo' = bo + Wo@bv,
  uses sum(attn_weights)=1), so V psum->SBUF is a pure copy on DVE.
- Softmax denominator via the ones column in packed v (positive now).
- One persistent 4-bank PSUM ring [128,2048] shared by scores/proj/outproj/
  bcast; exps run at N=2048 (two chunks per ACTIVATE).
- Projections are emitted as PE filler INSIDE the attention loop (pair pr
  emits Q/K of pair pr+2; V is emitted during pair 0) so the PE never idles
  while ScalarE runs exp, and ScalarE starts exp'ing ~10us into the kernel.
- Per-(pr,sq) normalization: reciprocal of the Z row straight out of PSUM
  (bf16), broadcast matmul, and the final scale runs on GpSimd.
"""

import os
import sys
from collections import deque

import numpy as np

for _p in ("/opt/trn_rl_repo",):
    if _p not in sys.path and os.path.isdir(_p):
        sys.path.insert(0, _p)

import ml_dtypes

import concourse.bass as bass
from concourse import bacc
import concourse.mybir as mybir
import concourse.tile as tile
from concourse.bass_utils import run_bass_kernel_spmd

BF16 = mybir.dt.bfloat16
F32 = mybir.dt.float32
U8 = mybir.dt.uint8
AF = mybir.ActivationFunctionType
MULT = mybir.AluOpType.mult

B, S, D, H, DH = 4, 2048, 1024, 16, 64
NCORES = 8
SH = S // 2
P = 128
NB = 512
KC = D // P          # 8 feature chunks
SKC = S // P         # 16 sk chunks
NPAIR = H // 2       # 8 head pairs == feature chunks
SQB = SH // NB       # 2 q blocks per core
VW = DH + 1          # v columns + ones column
VROW = H * VW

_bf16 = ml_dtypes.bfloat16


def _build_bass():
    nc = bacc.Bacc(num_devices=NCORES)

    qT_d = nc.declare_dram_parameter("qT", [D, S], BF16, isOutput=False)
    mu_d = nc.declare_dram_parameter("mu", [S, SH], U8, isOutput=False)
    wqT_d = nc.declare_dram_parameter("wqT", [D, D], BF16, isOutput=False)
    wkT_d = nc.declare_dram_parameter("wkT", [D, D], BF16, isOutput=False)
    wvT_d = nc.declare_dram_parameter("wvT", [D, D], BF16, isOutput=False)
    woT_d = nc.declare_dram_parameter("woT", [D, D], BF16, isOutput=False)
    bq_d = nc.declare_dram_parameter("bq", [D, 1], F32, isOutput=False)
    bk_d = nc.declare_dram_parameter("bk", [D, 1], F32, isOutput=False)
    bo_d = nc.declare_dram_parameter("bo", [D, 1], F32, isOutput=False)
    sel_d = nc.declare_dram_parameter("sel", [2, P], BF16, isOutput=False)
    out_d = nc.declare_dram_parameter("out", [D, SH], F32, isOutput=True)

    with tile.TileContext(nc) as tc:
        with (
            tc.tile_pool(name="persist", bufs=1) as persist,
            tc.tile_pool(name="qk", bufs=3) as qkpool,
            tc.tile_pool(name="apool", bufs=2) as apool,
            tc.tile_pool(name="work", bufs=2) as work,
            tc.tile_pool(name="psS", bufs=2, space="PSUM") as psS,
            tc.tile_pool(name="psV", bufs=2, space="PSUM") as psV,
        ):
            # ---------------- persistent SBUF ----------------
            qTb = persist.tile([P, KC * S], BF16)
            for kc in range(KC):
                nc.sync.dma_start(qTb[:, kc * S:(kc + 1) * S], qT_d[kc * P:(kc + 1) * P, :])
            wq = persist.tile([P, KC * D], BF16)
            wk = persist.tile([P, KC * D], BF16)
            wv = persist.tile([P, KC * D], BF16)
            for kc in range(KC):
                nc.sync.dma_start(wq[:, kc * D:(kc + 1) * D], wqT_d[kc * P:(kc + 1) * P, :])
                nc.sync.dma_start(wk[:, kc * D:(kc + 1) * D], wkT_d[kc * P:(kc + 1) * P, :])
                nc.sync.dma_start(wv[:, kc * D:(kc + 1) * D], wvT_d[kc * P:(kc + 1) * P, :])
            bq_sb = persist.tile([P, KC], F32)
            bk_sb = persist.tile([P, KC], F32)
            bo_sb = persist.tile([P, KC], F32)
            for mc in range(KC):
                nc.sync.dma_start(bq_sb[:, mc:mc + 1], bq_d[mc * P:(mc + 1) * P, :])
                nc.sync.dma_start(bk_sb[:, mc:mc + 1], bk_d[mc * P:(mc + 1) * P, :])
                nc.sync.dma_start(bo_sb[:, mc:mc + 1], bo_d[mc * P:(mc + 1) * P, :])
            sel_sb = persist.tile([2, P], BF16)
            nc.sync.dma_start(sel_sb[:], sel_d[:])
            recf = persist.tile([2, NB], BF16)
            nc.any.memset(recf[:], 1.0)
            sums = persist.tile([2, NB], F32)
            nc.any.memset(sums[:], 1.0)
            # inverted mask, [sk-part within chunk, (sq, chunk, q)]
            mTu = persist.tile([P, SQB * SKC * NB], U8)
            for sq in range(SQB):
                for c in range(SKC):
                    nc.sync.dma_start(
                        mTu[:, (sq * SKC + c) * NB:(sq * SKC + c + 1) * NB],
                        mu_d[c * P:(c + 1) * P, sq * NB:(sq + 1) * NB],
                    )
            ones1 = persist.tile([P, 4], BF16)
            nc.any.memset(ones1[:], 1.0)
            vpk = persist.tile([P, SKC * VROW], BF16)
            nc.any.memset(vpk[:], 1.0)   # ones columns; v parts overwritten
            outMT = persist.tile([P, KC * SH], BF16)

            def ps_tile():
                t = psS.tile([P, 2 * NB], F32, tag="ps", name="ps")
                return t

            # ---------------- projection emitters ----------------
            def emit_q(pr):
                t = qkpool.tile([P, SH], BF16, tag="qT", name="qTp")
                ps = ps_tile()
                for kc in range(KC):
                    w_sl = wq[:, kc * D + pr * P: kc * D + (pr + 1) * P]
                    nc.tensor.matmul(ps[:, 0:NB], w_sl, qTb[:, kc * S: kc * S + NB],
                                     start=(kc == 0), stop=(kc == KC - 1))
                    nc.tensor.matmul(ps[:, NB:2 * NB], w_sl, qTb[:, kc * S + NB: kc * S + 2 * NB],
                                     start=(kc == 0), stop=(kc == KC - 1))
                nc.scalar.activation(t[:], ps[:], AF.Identity, bias=bq_sb[:, pr:pr + 1])
                return t

            def emit_k_half(pr, t, nbp):
                ps = ps_tile()
                for kc in range(KC):
                    w_sl = wk[:, kc * D + pr * P: kc * D + (pr + 1) * P]
                    base = kc * S + nbp * 2 * NB
                    nc.tensor.matmul(ps[:, 0:NB], w_sl, qTb[:, base: base + NB],
                                     start=(kc == 0), stop=(kc == KC - 1))
                    nc.tensor.matmul(ps[:, NB:2 * NB], w_sl, qTb[:, base + NB: base + 2 * NB],
                                     start=(kc == 0), stop=(kc == KC - 1))
                nc.scalar.activation(t[:, nbp * 2 * NB:(nbp + 1) * 2 * NB], ps[:],
                                     AF.Identity, bias=bk_sb[:, pr:pr + 1])

            def emit_v(sc):
                ps = ps_tile()
                for kc in range(KC):
                    x_sl = qTb[:, kc * S + sc * P: kc * S + (sc + 1) * P]
                    nc.tensor.matmul(ps[:, 0:NB], x_sl, wv[:, kc * D: kc * D + NB],
                                     start=(kc == 0), stop=(kc == KC - 1))
                    nc.tensor.matmul(ps[:, NB:2 * NB], x_sl, wv[:, kc * D + NB: kc * D + 2 * NB],
                                     start=(kc == 0), stop=(kc == KC - 1))
                vdst3 = vpk[:, sc * VROW:(sc + 1) * VROW].rearrange("p (h w) -> p h w", h=H)
                eng = nc.vector if sc % 2 == 0 else nc.scalar
                if sc % 2 == 0:
                    nc.vector.tensor_copy(vdst3[:, :, 0:DH], ps.rearrange("p (h w) -> p h w", h=H))
                else:
                    nc.scalar.copy(vdst3[:, :, 0:DH], ps.rearrange("p (h w) -> p h w", h=H))

            # filler queue: (min_pair, closure).  Entries for pair pr may only
            # be emitted from pair pr-2 on (qk ring bufs=3 -> the ACT eviction
            # wait must target an already-finished pair).
            qts, kts = {}, {}
            filler = deque()
            for sc in range(SKC):
                filler.append((0, lambda sc=sc: emit_v(sc)))

            def queue_qk(pr):
                def do_q(pr=pr):
                    qts[pr] = emit_q(pr)
                def do_k0(pr=pr):
                    kts[pr] = qkpool.tile([P, S], BF16, tag="kT", name="kTp")
                    emit_k_half(pr, kts[pr], 0)
                def do_k1(pr=pr):
                    emit_k_half(pr, kts[pr], 1)
                filler.append((max(0, pr - 2), do_q))
                filler.append((max(0, pr - 2), do_k0))
                filler.append((max(0, pr - 2), do_k1))

            for pr in range(1, NPAIR):
                queue_qk(pr)

            def pop_filler(cur_pair, n):
                done = 0
                while done < n and filler and filler[0][0] <= cur_pair:
                    filler.popleft()[1]()
                    done += 1

            # prologue: pair 0 projections
            qts[0] = emit_q(0)
            kts[0] = qkpool.tile([P, S], BF16, tag="kT", name="kTp")
            emit_k_half(0, kts[0], 0)
            emit_k_half(0, kts[0], 1)

            # ---------------- attention ----------------
            norm_pending = None

            def finish_norm(pr, sq, za, zb):
                nc.vector.reciprocal_approx_fast(zfa_r[0:2, :], sums[:])
                with nc.allow_low_precision("bf16 softmax denominators"):
                    nc.vector.tensor_copy(recf[:], zfa_r[0:2, :])
                bcp = ps_tile()
                nc.tensor.matmul(bcp[0:DH, 0:NB], sel_sb[:, 0:DH], recf[:],
                                 start=True, stop=True)
                nc.tensor.matmul(bcp[0:DH, NB:2 * NB], sel_sb[:, DH:P], recf[:],
                                 start=True, stop=True)
                bc_sb = work.tile([P, 2 * NB], BF16, tag="bc", bufs=1)
                nc.scalar.copy(bc_sb[0:DH, :], bcp[0:DH, :])
                od = pr * SH + sq * NB
                nc.gpsimd.tensor_mul(
                    outMT[0:DH, od:od + NB], za[0:DH, :], bc_sb[0:DH, 0:NB])
                nc.gpsimd.tensor_mul(
                    outMT[DH:P, od:od + NB], zb[0:DH, :], bc_sb[0:DH, NB:2 * NB])

            zfa_r = work.tile([P, NB], F32, tag="zfr", bufs=1)
            for pr in range(NPAIR):
                qt, kt = qts[pr], kts[pr]
                for sq in range(SQB):
                    pv0 = psV.tile([P, NB], F32, tag="pv0")
                    pv1 = psV.tile([P, NB], F32, tag="pv1")

                    def emit_pv(g, a01g):
                        for c4 in range(4):
                            c = 4 * g + c4
                            for h01, pv in ((0, pv0), (1, pv1)):
                                hloc = 2 * pr + h01
                                nc.tensor.matmul(
                                    pv[0:VW, :],
                                    vpk[:, c * VROW + hloc * VW: c * VROW + (hloc + 1) * VW],
                                    a01g[:, c4 * 2 * NB + h01 * NB: c4 * 2 * NB + (h01 + 1) * NB],
                                    start=(c == 0), stop=(c == SKC - 1),
                                )

                    prev = None
                    for g in range(4):
                        a01g = apool.tile([P, 4 * 2 * NB], BF16, tag="a01")
                        for c4 in range(4):
                            c = 4 * g + c4
                            ps = ps_tile()
                            nc.tensor.matmul(
                                ps[:, 0:NB], kt[0:DH, c * P:(c + 1) * P],
                                qt[0:DH, sq * NB:(sq + 1) * NB],
                                start=True, stop=True, tile_position=(0, 0),
                            )
                            nc.tensor.matmul(
                                ps[:, NB:2 * NB], kt[DH:P, c * P:(c + 1) * P],
                                qt[DH:P, sq * NB:(sq + 1) * NB],
                                start=True, stop=True, tile_position=(64, 0),
                            )
                            nc.scalar.activation(
                                a01g[:, c4 * 2 * NB:(c4 + 1) * 2 * NB],
                                ps[:], AF.Exp, scale=0.125,
                            )
                        # patch masked slots to 1.0 (mask broadcast over head dup)
                        msl = mTu[:, (sq * SKC + 4 * g) * NB:(sq * SKC + 4 * g + 4) * NB]
                        m4 = msl.rearrange("p (c q) -> p c q", c=4).unsqueeze(2) \
                            .broadcast_to([P, 4, 2, NB])
                        o4 = ones1[:, 0:1].unsqueeze(2).unsqueeze(3) \
                            .broadcast_to([P, 4, 2, NB])
                        nc.vector.copy_predicated(
                            a01g[:].rearrange("p (c d q) -> p c d q", c=4, d=2),
                            m4, o4,
                        )
                        if prev is not None:
                            emit_pv(*prev)
                        pop_filler(pr, 4 if (pr == 0 and sq == 0) else 2)
                        prev = (g, a01g)
                    emit_pv(*prev)

                    # -------- normalization, software-pipelined by one unit:
                    # copy out + fire Z DMAs for THIS unit; finish (recip,
                    # broadcast, scale) for the PREVIOUS unit whose Z landed
                    # ~40us ago, so nothing in the ACT/DVE FIFOs blocks on a
                    # fresh DMA round-trip.
                    za = work.tile([P, NB], BF16, tag="za")
                    zb = work.tile([P, NB], BF16, tag="zb")
                    zfa = work.tile([P, NB], F32, tag="zfa", bufs=1)
                    nc.scalar.copy(za[0:VW, :], pv0[0:VW, :])
                    nc.scalar.copy(zb[0:VW, :], pv1[0:VW, :])
                    nc.vector.tensor_copy(zfa[DH:VW, :], pv0[DH:VW, :])
                    nc.vector.tensor_copy(zfa[96:97, :], pv1[DH:VW, :])
                    if norm_pending is not None:
                        finish_norm(*norm_pending)
                    nc.sync.dma_start(sums[0:1, :], zfa[DH:VW, :])
                    nc.sync.dma_start(sums[1:2, :], zfa[96:97, :])
                    norm_pending = (pr, sq, za, zb)

            if norm_pending is not None:
                finish_norm(*norm_pending)
                norm_pending = None

            # ---------------- output projection ----------------
            wo_rings = []
            for mc in range(KC):
                wo_mc = work.tile([P, KC * P], BF16, tag="womc", bufs=3, name="wo_mc")
                for kc in range(KC):
                    nc.sync.dma_start(wo_mc[:, kc * P:(kc + 1) * P],
                                      woT_d[kc * P:(kc + 1) * P, mc * P:(mc + 1) * P])
                wo_rings.append(wo_mc)
            for mc in range(KC):
                wo_mc = wo_rings[mc]
                ps = ps_tile()
                for kc in range(KC):
                    w_sl = wo_mc[:, kc * P:(kc + 1) * P]
                    nc.tensor.matmul(ps[:, 0:NB], w_sl, outMT[:, kc * SH: kc * SH + NB],
                                     start=(kc == 0), stop=(kc == KC - 1))
                    nc.tensor.matmul(ps[:, NB:2 * NB], w_sl,
                                     outMT[:, kc * SH + NB: kc * SH + 2 * NB],
                                     start=(kc == 0), stop=(kc == KC - 1))
                for nb in range(2):
                    fin = work.tile([P, NB], F32, tag="fin", bufs=1)
                    nc.scalar.activation(fin[:], ps[:, nb * NB:(nb + 1) * NB],
                                         AF.Identity, bias=bo_sb[:, mc:mc + 1])
                    nc.sync.dma_start(out_d[mc * P:(mc + 1) * P, nb * NB:(nb + 1) * NB],
                                      fin[:])

    nc.finalize()
    return nc


_NC_CACHE = None
LAST_RESULTS = None


def _get_nc():
    global _NC_CACHE
    if _NC_CACHE is None:
        _NC_CACHE = _build_bass()
    return _NC_CACHE


def kernel(query, mask, Wq, bq, Wk, bk, Wv, bv, Wo, bo, **_unused):
    query = np.asarray(query, dtype=np.float32)
    mask = np.asarray(mask).astype(bool)
    Wq = np.asarray(Wq, dtype=np.float32)
    Wk = np.asarray(Wk, dtype=np.float32)
    Wv = np.asarray(Wv, dtype=np.float32)
    Wo = np.asarray(Wo, dtype=np.float32)
    bq = np.asarray(bq, dtype=np.float32)
    bk = np.asarray(bk, dtype=np.float32)
    bv = np.asarray(bv, dtype=np.float32)
    bo = np.asarray(bo, dtype=np.float32)

    wqT = np.ascontiguousarray(Wq.T).astype(_bf16)
    wkT = np.ascontiguousarray(Wk.T).astype(_bf16)
    wvT = np.ascontiguousarray(Wv.T).astype(_bf16)
    woT = np.ascontiguousarray(Wo.T).astype(_bf16)
    bq_c = np.ascontiguousarray(bq.reshape(D, 1))
    bk_c = np.ascontiguousarray(bk.reshape(D, 1))
    # V bias folded through the output projection (sum of attn weights == 1)
    bo_c = np.ascontiguousarray((bo + Wo @ bv).reshape(D, 1))
    sel_np = np.zeros((2, P), dtype=np.float32)
    sel_np[0, 0:DH] = 1.0      # head-0 selector: stationary cols 0:64
    sel_np[1, DH:P] = 1.0      # head-1 selector: stationary cols 64:128
    sel_bf = sel_np.astype(_bf16)

    in_maps = []
    for c in range(NCORES):
        b, half = c // 2, c % 2
        off = half * SH
        qT_rot = np.ascontiguousarray(np.roll(query[b].T, -off, axis=1)).astype(_bf16)
        minv = np.roll((~mask[b]).T, -off, axis=0)      # [sk, q], True where masked
        mu8 = np.ascontiguousarray(minv[:, off:off + SH]).astype(np.uint8)
        in_maps.append({
            "qT": qT_rot, "mu": mu8,
            "wqT": wqT, "wkT": wkT, "wvT": wvT, "woT": woT,
            "bq": bq_c, "bk": bk_c, "bo": bo_c, "sel": sel_bf,
            "out": np.zeros((D, SH), dtype=np.float32),
        })

    nc = _get_nc()
    res = run_bass_kernel_spmd(nc, in_maps, core_ids=list(range(NCORES)))
    global LAST_RESULTS
    LAST_RESULTS = res

    out = np.empty((B, S, D), dtype=np.float32)
    for c in range(NCORES):
        b, half = c // 2, c % 2
        out[b, half * SH:(half + 1) * SH, :] = res.results[c]["out"].T
    return out
